# revision 1
# baseline (speedup 1.0000x reference)
"""Trainium2 Bass kernel for a DiT block (AdaRMSNorm + MHA + AdaRMSNorm + SwiGLU).

Sharding: 8 cores = 4 batches x 2 query-halves.  Each core owns 1024 query
tokens of one batch: it computes K/V over the full 2048 tokens of its batch
(redundantly with its pair core, ~10% extra FLOPs) and everything else only
for its own 1024 tokens.  Zero collectives.

On-chip layout is "transposed": activations live as [d, tokens] so all GEMMs
contract over the partition axis and per-channel modulations (gamma/beta/
alpha/bias) are native per-partition scalars.  Matmuls run in bf16 (weights
pre-cast on host), statistics/residual stream in fp32.
"""

import numpy as np

P = 128
D = 1024
DT = 256
DH = 4096
NH = 16
L = 2048
LOWN = 1024
EPS = 1e-6
SM_SCALE = 0.125  # sqrt(n_heads / d_model) = 1/sqrt(d_head)
NCORES = 8

_CACHE = {}


def _build_nc():
    from contextlib import ExitStack
    import os
    _SIM_COMPAT = bool(int(os.environ.get("KERNEL_SIM_COMPAT", "0")))
    _PHASES = int(os.environ.get("KERNEL_PHASES", "5"))

    import concourse.bass as bass  # noqa: F401
    import concourse.tile as tile
    from concourse import bacc, mybir

    f32 = mybir.dt.float32
    bf16 = mybir.dt.bfloat16
    AF = mybir.ActivationFunctionType
    ALU = mybir.AluOpType

    nc = bacc.Bacc("TRN2", target_bir_lowering=False, debug=False,
                   num_devices=NCORES)

    # ---- DRAM I/O ----
    xbT = nc.dram_tensor("xbT", [D, L], f32, kind="ExternalInput").ap()
    tb = nc.dram_tensor("tb", [P, 2], bf16, kind="ExternalInput").ap()
    modw = nc.dram_tensor("modw", [DT, 6 * D], bf16, kind="ExternalInput").ap()
    wq = nc.dram_tensor("wq", [D, D], bf16, kind="ExternalInput").ap()
    wk = nc.dram_tensor("wk", [D, D], bf16, kind="ExternalInput").ap()
    wv = nc.dram_tensor("wv", [D, D], bf16, kind="ExternalInput").ap()
    wo = nc.dram_tensor("wo", [D, D], bf16, kind="ExternalInput").ap()
    wg = nc.dram_tensor("wg", [D, DH], bf16, kind="ExternalInput").ap()
    wh = nc.dram_tensor("wh", [D, DH], bf16, kind="ExternalInput").ap()
    wout = nc.dram_tensor("wout", [DH, D], bf16, kind="ExternalInput").ap()
    outb = nc.dram_tensor("outb", [P, 8], f32, kind="ExternalInput").ap()
    y = nc.dram_tensor("y", [D, LOWN], f32, kind="ExternalOutput").ap()

    xbT_v = xbT.rearrange("(o p) t -> p o t", p=P)      # [128, 8, 2048]
    modw_v = modw.rearrange("(c p) n -> p c n", p=P)    # [128, 2, 6144]
    wq_v = wq.rearrange("(o p) n -> p o n", p=P)        # [128, 8, 1024]
    wk_v = wk.rearrange("(o p) n -> p o n", p=P)
    wv_v = wv.rearrange("(o p) n -> p o n", p=P)
    wo_v = wo.rearrange("(o p) n -> p o n", p=P)
    wg_v = wg.rearrange("(o p) n -> p o n", p=P)        # [128, 8, 4096]
    wh_v = wh.rearrange("(o p) n -> p o n", p=P)
    wout_v = wout.rearrange("(o p) n -> p o n", p=P)    # [128, 32, 1024]
    y_v = y.rearrange("(o p) t -> p o t", p=P)          # [128, 8, 1024]

    with tile.TileContext(nc) as tc, ExitStack() as top:
        TPool = tc.tile_pool
        constp = top.enter_context(TPool(name="const", bufs=1))
        ones_bf = constp.tile([P, 1], bf16, name="ones_bf")
        nc.vector.memset(ones_bf[:], 1.0)
        eps_sb = constp.tile([P, 1], f32, name="eps_sb")
        nc.vector.memset(eps_sb[:], EPS)
        tb_sb = constp.tile([P, 2], bf16, name="tb_sb")
        nc.sync.dma_start(tb_sb[:], tb)
        outb_sb = constp.tile([P, 8], f32, name="outb_sb")
        # modulation vectors: col j*8+c is (vector j, d-chunk c); j order:
        # attn_gamma, attn_beta, attn_alpha, ffn_gamma, ffn_beta, ffn_alpha
        mods = constp.tile([P, 48], f32, name="mods")

        # ---------- phase 0: modulation vectors ----------
        def emit_mods(p0ps, modw_sb, ch_range, ch0):
            for ch in ch_range:  # 48 cols in groups of 4
                pc = p0ps.tile([P, 4], f32, tag="pc", name=f"pc{ch}")
                for g in range(4):
                    m = (ch - ch0) * 4 + g
                    for kc in range(2):
                        nc.tensor.matmul(
                            pc[:, g:g + 1],
                            lhsT=modw_sb[:, kc, m * P:(m + 1) * P],
                            rhs=tb_sb[:, kc:kc + 1],
                            start=(kc == 0), stop=(kc == 1))
                nc.vector.tensor_copy(mods[:, ch * 4:(ch + 1) * 4], pc[:])

        # ---------- persistent attention tensors ----------
        persA = tc.alloc_tile_pool(name="persA", bufs=1)
        kT = persA.tile([P, 8, L], bf16, name="kT")         # [d-chunk, k-tok]
        vA = persA.tile([P, 16, NH * 65], bf16, name="vA")  # v_aug per k-chunk
        qT = persA.tile([P, 8, LOWN], bf16, name="qT")

        # ---------- phase 1: attn AdaRMSNorm + QKV projections ----------
        TB = 256
        NBLK = L // TB
        with TPool(name="p0", bufs=1) as p0, \
             TPool(name="p0ps", bufs=2, space="PSUM") as p0ps, \
             TPool(name="p1w", bufs=1) as p1w, \
             TPool(name="p1x", bufs=3) as p1x, \
             TPool(name="p1s", bufs=6) as p1s, \
             TPool(name="p1n", bufs=4) as p1n, \
             TPool(name="p1r", bufs=3) as p1r, \
             TPool(name="p1ps_s", bufs=2, space="PSUM") as p1ps_s, \
             TPool(name="p1ps_a", bufs=3, space="PSUM") as p1ps_a, \
             TPool(name="p1ps_b", bufs=1, space="PSUM") as p1ps_b:
            wq_sb = p1w.tile([P, 8, D], bf16, name="wq_sb")
            wk_sb = p1w.tile([P, 8, D], bf16, name="wk_sb")
            wv_sb = p1w.tile([P, 8, D], bf16, name="wv_sb")
            modw_att = p0.tile([P, 2, 2 * D], bf16, name="modw_att")
            modw_rest = p0.tile([P, 2, 4 * D], bf16, name="modw_rest")
            xtiles = {}

            def load_x(blk):
                t = p1x.tile([P, 8, TB], f32, tag="xblk", name=f"xblk{blk}")
                nc.sync.dma_start(t[:], xbT_v[:, :, blk * TB:(blk + 1) * TB])
                xtiles[blk] = t

            # DMA priority order (transfers serialize on the DMA engines,
            # and cross-queue round-robin defeats ordering -> single queue).
            load_x(0)
            nc.sync.dma_start(modw_att[:], modw_v[:, :, 0:2 * D])
            nc.sync.dma_start(wk_sb[:, :, 0:512], wk_v[:, :, 0:512])
            nc.sync.dma_start(wk_sb[:, :, 512:D], wk_v[:, :, 512:D])
            emit_mods(p0ps, modw_att, range(4), 0)   # attn gamma/beta
            load_x(1)
            nc.sync.dma_start(wq_sb[:, :, 0:512], wq_v[:, :, 0:512])
            nc.sync.dma_start(wq_sb[:, :, 512:D], wq_v[:, :, 512:D])
            nc.sync.dma_start(wv_sb[:, :, 0:512], wv_v[:, :, 0:512])
            nc.sync.dma_start(wv_sb[:, :, 512:D], wv_v[:, :, 512:D])

            # ones columns of v_aug (col h*65+64 = 1)
            nc.vector.memset(
                vA.rearrange("p c (h e) -> p c h e", e=65)[:, :, :, 64:65], 1.0)

            for blk in range(NBLK):  # token blocks (first half = own q)
                tsl = slice(blk * TB, (blk + 1) * TB)
                xblk = xtiles.pop(blk)
                if blk + 2 < NBLK:
                    load_x(blk + 2)
                if blk == 1:
                    # remaining modulation vectors (alpha + ffn), off the
                    # critical path
                    nc.sync.dma_start(modw_rest[:],
                                      modw_v[:, :, 2 * D:6 * D])
                    emit_mods(p0ps, modw_rest, range(4, 12), 4)
                # rms statistics: sum_d x^2 via ones-matmul on bf16 squares
                ps_s = p1ps_s.tile([1, TB], f32, tag="ps_s", name=f"pss{blk}")
                for o in range(8):
                    sq = p1s.tile([P, TB], bf16, tag="sq", name=f"sq{blk}_{o}")
                    nc.scalar.activation(sq[:], xblk[:, o, :], AF.Square)
                    nc.tensor.matmul(ps_s[:], lhsT=ones_bf[:], rhs=sq[:],
                                     start=(o == 0), stop=(o == 7))
                # r = 1/sqrt(mean + eps), broadcast to 128 partitions
                srow = p1r.tile([1, TB], f32, tag="srow", name=f"srow{blk}")
                nc.scalar.activation(srow[:], ps_s[:], AF.Sqrt,
                                     scale=1.0 / D, bias=eps_sb[0:1, :])
                rrow = p1r.tile([1, TB], f32, tag="rrow", name=f"rrow{blk}")
                nc.vector.reciprocal(rrow[:], srow[:])
                rbc = p1r.tile([P, TB], f32, tag="rbc", name=f"rbc{blk}")
                nc.gpsimd.partition_broadcast(rbc[:], rrow[:])
                # xn = gamma * (x * r) + beta   (bf16)
                xn = p1n.tile([P, 8, TB], bf16, tag="xn", name=f"xn{blk}")
                for o in range(8):
                    nc.vector.scalar_tensor_tensor(
                        xn[:, o, :], xblk[:, o, :], mods[:, o:o + 1], rbc[:],
                        op0=ALU.mult, op1=ALU.mult)
                    nc.vector.tensor_scalar_add(xn[:, o, :], xn[:, o, :],
                                                mods[:, 8 + o:9 + o])
                # K projection (all blocks) and Q projection (own blocks)
                for dst, w_sb, own in ((kT, wk_sb, False), (qT, wq_sb, True)):
                    if own and blk >= LOWN // TB:
                        continue
                    for m in range(8):
                        pp = p1ps_a.tile([P, TB], f32, tag="ppa",
                                         name=f"pa{blk}_{m}")
                        for o in range(8):
                            nc.tensor.matmul(
                                pp[:], lhsT=w_sb[:, o, m * P:(m + 1) * P],
                                rhs=xn[:, o, :],
                                start=(o == 0), stop=(o == 7))
                        nc.vector.tensor_copy(dst[:, m, tsl], pp[:])
                # V projection -> natural layout with per-head ones gap
                for mt in range(TB // P):
                    kcg = blk * (TB // P) + mt  # global k-token chunk
                    for half in range(2):
                        pv = p1ps_b.tile([P, 512], f32, tag="ppv",
                                         name=f"pv{blk}_{mt}_{half}")
                        for o in range(8):
                            nc.tensor.matmul(
                                pv[:],
                                lhsT=xn[:, o, mt * P:(mt + 1) * P],
                                rhs=wv_sb[:, o, half * 512:(half + 1) * 512],
                                start=(o == 0), stop=(o == 7))
                        dst = vA.rearrange("p c (h e) -> p c h e", e=65)[
                            :, kcg, half * 8:(half + 1) * 8, 0:64]
                        nc.vector.tensor_copy(
                            dst, pv.rearrange("p (h e) -> p h e", e=64))

        # ---------- persistent post-attention tensors ----------
        persN0 = tc.alloc_tile_pool(name="persN0", bufs=1, side="right")
        xn2a = persN0.tile([P, 8, 512], bf16, name="xn2a")
        persB = tc.alloc_tile_pool(name="persB", bufs=1, side="right")
        oT = persB.tile([P, 8, LOWN], bf16, name="oT")
        wo_sb = persB.tile([P, 8, D], bf16, name="wo_sb")
        nc.sync.dma_start(wo_sb[:], wo_v)

        # ---------- phase 2: attention (per head pair, per query half) ----
        p3ps_y = tc.alloc_tile_pool(name="p3ps_y", bufs=2, space="PSUM")
        with TPool(name="p2e", bufs=2) as p2e, \
             TPool(name="p2r", bufs=2) as p2r, \
             TPool(name="p2ps_s", bufs=2, space="PSUM") as p2ps_s, \
             TPool(name="p2ps_o", bufs=2, space="PSUM") as p2ps_o:
            for hp in range(8 if _PHASES >= 2 else 0):
                for qh in range(2):  # query half (512)
                    qsl = slice(qh * 512, (qh + 1) * 512)
                    exps = [p2e.tile([P, 16, 512], bf16, tag=f"exp{ab}",
                                     name=f"exp{ab}_{hp}_{qh}")
                            for ab in range(2)]
                    pos = [p2ps_o.tile([65, 512], f32, tag="psav",
                                       name=f"po{hp}_{qh}_{ab}")
                           for ab in range(2)]
                    for kc2 in range(8):  # pairs of k-chunks share a psum tile
                        for ab in range(2):
                            ps = p2ps_s.tile([P, 2, 512], f32, tag="pssc",
                                             name=f"ps{hp}_{qh}_{kc2}_{ab}")
                            rows = slice(ab * 64, ab * 64 + 64)
                            for j in range(2):
                                kc = kc2 * 2 + j
                                nc.tensor.matmul(
                                    ps[:, j, :],
                                    lhsT=kT[rows, hp, kc * P:(kc + 1) * P],
                                    rhs=qT[rows, hp, qsl],
                                    start=True, stop=True)
                            nc.scalar.activation(
                                exps[ab][:, kc2 * 2:kc2 * 2 + 2, :], ps[:],
                                AF.Exp, scale=SM_SCALE)
                        for ab in range(2):
                            h = 2 * hp + ab
                            for j in range(2):
                                kc = kc2 * 2 + j
                                nc.tensor.matmul(
                                    pos[ab][:],
                                    lhsT=vA[:, kc, h * 65:h * 65 + 65],
                                    rhs=exps[ab][:, kc, :],
                                    start=(kc == 0), stop=(kc == 15))
                    for ab in range(2):
                        po = pos[ab]
                        rec = p2r.tile([1, 512], f32, tag="rec",
                                       name=f"rec{hp}_{qh}_{ab}")
                        nc.vector.reciprocal(rec[:], po[64:65, :])
                        bc = p2r.tile([64, 512], f32, tag="bc",
                                      name=f"bc{hp}_{qh}_{ab}")
                        nc.gpsimd.partition_broadcast(bc[:], rec[:])
                        nc.vector.tensor_tensor(
                            oT[ab * 64:ab * 64 + 64, hp, qsl],
                            po[0:64, :], bc[:], ALU.mult)

        persA.release()  # free kT, vA, qT
        p4wg = tc.alloc_tile_pool(name="p4wg", bufs=2)
        p4wh = tc.alloc_tile_pool(name="p4wh", bufs=2)

        # ---------- persistent residual tensors ----------
        persBx = tc.alloc_tile_pool(name="persBx", bufs=1, side="right")
        xown = persBx.tile([P, 8, LOWN], f32, name="xown")
        nc.sync.dma_start(xown[:], xbT_v[:, :, 0:LOWN])
        persC = tc.alloc_tile_pool(name="persC", bufs=1)
        x2 = persC.tile([P, 8, LOWN], f32, name="x2")

        # ---------- phase 3: O-proj + residual + ffn AdaRMSNorm ----------
        # Stage A: partial O-proj over head-pairs 0-3, which finish early in
        # phase 2 -- these matmuls fill the PE drain while the last pairs'
        # exp/AV straggle.  Stage B adds pairs 4-7 and applies the residual.
        def emit_oproj_a(nh2, pa_pool):
            tsl = slice(nh2 * 512, (nh2 + 1) * 512)
            pas = []
            for m in range(8):
                py = p3ps_y.tile([P, 512], f32, tag="py",
                                 name=f"pyA{m}_{nh2}")
                for o in range(4):
                    nc.tensor.matmul(
                        py[:], lhsT=wo_sb[:, o, m * P:(m + 1) * P],
                        rhs=oT[:, o, tsl],
                        start=(o == 0), stop=(o == 3))
                pa = pa_pool.tile([P, 512], f32, tag=f"pa{nh2}_{m}",
                                  name=f"pa{nh2}_{m}")
                nc.vector.tensor_copy(pa[:], py[:])
                pas.append(pa)
            return pas

        def emit_oproj_b(nh2, pas, ps_s, p3s):
            tsl = slice(nh2 * 512, (nh2 + 1) * 512)
            for m in range(8):
                py = p3ps_y.tile([P, 512], f32, tag="py",
                                 name=f"pyB{m}_{nh2}")
                for o in range(4, 8):
                    nc.tensor.matmul(
                        py[:], lhsT=wo_sb[:, o, m * P:(m + 1) * P],
                        rhs=oT[:, o, tsl],
                        start=(o == 4), stop=(o == 7))
                pa = pas[m]
                nc.vector.tensor_tensor(pa[:], py[:], pa[:], ALU.add)
                # x2 = xown + a_alpha * o_proj
                nc.vector.scalar_tensor_tensor(
                    x2[:, m, tsl], pa[:], mods[:, 16 + m:17 + m],
                    xown[:, m, tsl], op0=ALU.mult, op1=ALU.add)
                # ffn rms statistics, interleaved
                sq = p3s.tile([P, 512], bf16, tag="sq2",
                              name=f"sq2{nh2}_{m}")
                nc.scalar.activation(sq[:], x2[:, m, tsl], AF.Square)
                nc.tensor.matmul(ps_s[:], lhsT=ones_bf[:], rhs=sq[:],
                                 start=(m == 0), stop=(m == 7))

        def emit_norm_half(nh2, ps_s, xt, p3r):
            tsl = slice(nh2 * 512, (nh2 + 1) * 512)
            srow = p3r.tile([1, 512], f32, tag="srow2", name=f"sr2{nh2}")
            nc.scalar.activation(srow[:], ps_s[:], AF.Sqrt,
                                 scale=1.0 / D, bias=eps_sb[0:1, :])
            rrow = p3r.tile([1, 512], f32, tag="rrow2", name=f"rr2{nh2}")
            nc.vector.reciprocal(rrow[:], srow[:])
            rbc = p3r.tile([P, 512], f32, tag="rbc2", name=f"rbc2{nh2}")
            nc.gpsimd.partition_broadcast(rbc[:], rrow[:])
            for o in range(8):
                nc.vector.scalar_tensor_tensor(
                    xt[:, o, :], x2[:, o, tsl], mods[:, 24 + o:25 + o],
                    rbc[:], op0=ALU.mult, op1=ALU.mult)
                nc.vector.tensor_scalar_add(xt[:, o, :], xt[:, o, :],
                                            mods[:, 32 + o:33 + o])

        with TPool(name="p3ps_s", bufs=2, space="PSUM") as p3ps_s, \
             TPool(name="p3r", bufs=2) as p3r:
            ps_s2 = [p3ps_s.tile([1, 512], f32, tag="ps2", name=f"ps2{nh}")
                     for nh in range(2 if _PHASES >= 3 else 0)]
            with TPool(name="p3s", bufs=2) as p3s, \
                 TPool(name="p3pa", bufs=1) as p3pa:
                if _PHASES >= 3:
                    pas0 = emit_oproj_a(0, p3pa)
                    pas1 = emit_oproj_a(1, p3pa)
                    emit_oproj_b(0, pas0, ps_s2[0], p3s)
                    # half-0 norm chain hides under half-1's projection
                    emit_norm_half(0, ps_s2[0], xn2a, p3r)
                    emit_oproj_b(1, pas1, ps_s2[1], p3s)

            persBx.release()  # free xown
            persB.release()   # free oT, wo_sb
            persN1 = tc.alloc_tile_pool(name="persN1", bufs=1, side="right")
            xn2b = persN1.tile([P, 8, 512], bf16, name="xn2b")
            if _PHASES >= 3:
                emit_norm_half(1, ps_s2[1], xn2b, p3r)
        p3ps_y.release()
        persD = tc.alloc_tile_pool(name="persD", bufs=1)
        m_sb = persD.tile([P, 32, LOWN], bf16, name="m_sb")
        p5w = tc.alloc_tile_pool(name="p5w", bufs=12)

        # ---------- phase 4: SwiGLU up (m = silu(g) * h) ----------
        nc.sync.dma_start(outb_sb[:], outb)
        with TPool(name="p4s", bufs=4) as p4s, \
             TPool(name="p4ps_g", bufs=2, space="PSUM") as p4ps_g, \
             TPool(name="p4ps_h", bufs=2, space="PSUM") as p4ps_h:
            for hb in range(8 if _PHASES >= 4 else 0):
                hsl = slice(hb * 512, (hb + 1) * 512)
                wg_sb = p4wg.tile([P, 8, 512], bf16, tag="wg", name=f"wg{hb}")
                wh_sb = p4wh.tile([P, 8, 512], bf16, tag="wh", name=f"wh{hb}")
                nc.sync.dma_start(wg_sb[:], wg_v[:, :, hsl])
                nc.sync.dma_start(wh_sb[:], wh_v[:, :, hsl])
                for mt in range(4):
                    mi = hb * 4 + mt  # global hidden chunk (of 32)
                    for nh2 in range(2):
                        tsl = slice(nh2 * 512, (nh2 + 1) * 512)
                        pg = p4ps_g.tile([P, 512], f32, tag="pg",
                                         name=f"pg{mi}_{nh2}")
                        ph = p4ps_h.tile([P, 512], f32, tag="ph",
                                         name=f"ph{mi}_{nh2}")
                        xnh = xn2a if nh2 == 0 else xn2b
                        for o in range(8):
                            nc.tensor.matmul(
                                pg[:], lhsT=wg_sb[:, o, mt * P:(mt + 1) * P],
                                rhs=xnh[:, o, :],
                                start=(o == 0), stop=(o == 7))
                        for o in range(8):
                            nc.tensor.matmul(
                                ph[:], lhsT=wh_sb[:, o, mt * P:(mt + 1) * P],
                                rhs=xnh[:, o, :],
                                start=(o == 0), stop=(o == 7))
                        sg = p4s.tile([P, 512], bf16, tag="sg",
                                      name=f"sg{mi}_{nh2}")
                        if _SIM_COMPAT:
                            nc.scalar.activation(sg[:], pg[:], AF.Sigmoid)
                            gs = p4s.tile([P, 512], bf16, tag="gs",
                                          name=f"gs{mi}_{nh2}")
                            nc.vector.tensor_tensor(gs[:], pg[:], sg[:],
                                                    ALU.mult)
                        else:
                            nc.scalar.activation(sg[:], pg[:], AF.Silu)
                            gs = sg
                        nc.vector.tensor_tensor(
                            m_sb[:, mi, tsl], ph[:], gs[:], ALU.mult)
        persN1.release()  # free xn2b
        persN0.release()  # free xn2a

        # ---------- phase 5: down-projection + bias + residual ----------
        with TPool(name="p5z", bufs=4) as p5z, \
             TPool(name="p5y", bufs=4) as p5y, \
             TPool(name="p5ps", bufs=8, space="PSUM") as p5ps:
            for nh2 in range(2 if _PHASES >= 5 else 0):
                tsl = slice(nh2 * 512, (nh2 + 1) * 512)
                pys = [p5ps.tile([P, 512], f32, tag="pyd",
                                 name=f"pyd{nh2}_{m}") for m in range(8)]
                for kb in range(32):
                    wkb = p5w.tile([P, D], bf16, tag="wkb",
                                   name=f"wkb{nh2}_{kb}")
                    nc.sync.dma_start(wkb[:], wout_v[:, kb, :])
                    for m in range(8):
                        nc.tensor.matmul(
                            pys[m], lhsT=wkb[:, m * P:(m + 1) * P],
                            rhs=m_sb[:, kb, tsl],
                            start=(kb == 0), stop=(kb == 31))
                for m in range(8):
                    z = p5z.tile([P, 512], f32, tag="z", name=f"z{nh2}_{m}")
                    nc.scalar.activation(z[:], pys[m], AF.Identity,
                                         bias=outb_sb[:, m:m + 1])
                    yt = p5y.tile([P, 512], f32, tag="yt",
                                  name=f"yt{nh2}_{m}")
                    nc.vector.scalar_tensor_tensor(
                        yt[:], z[:], mods[:, 40 + m:41 + m], x2[:, m, tsl],
                        op0=ALU.mult, op1=ALU.add)
                    nc.sync.dma_start(y_v[:, m, tsl], yt[:])
        p5w.release()
        persD.release()
        persC.release()
        p4wh.release()
        p4wg.release()

    nc.compile()
    return nc


def _get_nc():
    if "nc" not in _CACHE:
        _CACHE["nc"] = _build_nc()
    return _CACHE["nc"]


def make_in_maps(x, t, attn_gamma_w, attn_beta_w, W_q, W_k, W_v, W_o,
                 attn_alpha_w, ffn_gamma_w, ffn_beta_w, gate_w, hidden_w,
                 out_w, out_b, ffn_alpha_w):
    import ml_dtypes
    bf = ml_dtypes.bfloat16
    f32 = np.float32

    def T(a):
        return np.ascontiguousarray(np.asarray(a, f32).T)

    xT = np.ascontiguousarray(np.asarray(x, f32).transpose(0, 2, 1))
    t = np.asarray(t, f32)
    modw = np.ascontiguousarray(np.concatenate(
        [np.asarray(w, f32) for w in (attn_gamma_w, attn_beta_w, attn_alpha_w,
                                      ffn_gamma_w, ffn_beta_w, ffn_alpha_w)],
        axis=0).T).astype(bf)                          # [256, 6144]
    shared = {
        "modw": modw,
        "wq": T(W_q).astype(bf), "wk": T(W_k).astype(bf),
        "wv": T(W_v).astype(bf), "wo": T(W_o).astype(bf),
        "wg": T(gate_w).astype(bf), "wh": T(hidden_w).astype(bf),
        "wout": T(out_w).astype(bf),
        "outb": np.ascontiguousarray(np.asarray(out_b, f32).reshape(8, P).T),
    }
    in_maps = []
    for c in range(NCORES):
        b, h = c // 2, c % 2
        if h == 0:
            xbT = xT[b]
        else:
            xbT = np.concatenate([xT[b][:, LOWN:], xT[b][:, :LOWN]], axis=1)
        in_maps.append(dict(
            shared,
            xbT=np.ascontiguousarray(xbT),
            tb=np.ascontiguousarray(t[b].reshape(2, P).T).astype(bf),
        ))
    return in_maps


def kernel(**inputs):
    from concourse.bass_utils import run_bass_kernel_spmd

    nc = _get_nc()
    in_maps = make_in_maps(**inputs)
    res = run_bass_kernel_spmd(nc, in_maps, core_ids=list(range(NCORES)))
    x = np.asarray(inputs["x"])
    yfull = np.empty((x.shape[0], L, D), dtype=np.float32)
    for c in range(NCORES):
        b, h = c // 2, c % 2
        yfull[b, h * LOWN:(h + 1) * LOWN, :] = res.results[c]["y"].T
    return yfull



# revision 4
# speedup vs baseline: 1.5617x; 1.5617x over previous
"""Trainium2 Bass kernel for a DiT block (AdaRMSNorm + MHA + AdaRMSNorm + SwiGLU).

Sharding: 8 cores = 4 batches x 2 query-halves (as the baseline).  Each core
owns 1024 query tokens of one batch; K/V over the full 2048 tokens of its
batch; zero collectives.

Speed strategy vs the bf16 baseline:
  - All projection/AV GEMMs run in fp8e4 with DoubleRow perf mode (2 k-tiles
    contracted per instruction at 0.5 cycles/row).  Scores run in plain fp8.
  - The block is pipelined over 4 query-chunks of 256: window w runs
    attention (scores -> exp -> AV -> oT) for chunk w interleaved with the
    complete FFN of chunk w-1, keeping PE/DVE/Pool busy under the
    Activation-engine exp stream.
  - exp for a subset of heads runs on the Pool engine as a quadratic
    e' = s'(1 + s'/2) (logits are within [-0.5, 0.5]); the missing "+1" is
    recovered exactly through per-head V-sum columns and a +2048 denominator
    shift, since softmax weights sum to one.
  - silu is computed via Exp (same ACT table as attention exp, so the ACT
    engine never reloads activation tables): silu(g)*h = g*h/(1+e^-g).
  - AdaRMSNorm betas for the attention branch are folded into the PSUM
    evacuations (K/Q via ACT bias; V exactly via Wo@(Wv beta) at the
    O-projection).  ffn-norm rsqrt runs on DVE with 2 Newton iterations.
  - wh streams per window; wg/wout/wo stay resident (SBUF budget).
"""

import numpy as np

P = 128
D = 1024
DT = 256
DH = 4096
NH = 16
L = 2048
LOWN = 1024
QC = 256
NQC = 4
TB = 256
NBLK = 8
EPS = 1e-6
SM = 0.125  # 1/sqrt(d_head)
NCORES = 8

_CACHE = {}


def _build_nc():
    from contextlib import ExitStack
    import os
    _POOL_HEADS = int(os.environ.get("KERNEL_POOL_HEADS", "0"))
    # heads whose score-groups 1 and 3 take the Pool-quad softmax path
    pool_set = set(range(1, 2 * min(_POOL_HEADS, 16), 2))[:_POOL_HEADS] \
        if False else set(list(range(1, 16, 2))[:_POOL_HEADS])

    import concourse.bass as bass  # noqa: F401
    import concourse.tile as tile
    from concourse import bacc, mybir
    from concourse import bass_isa

    f32 = mybir.dt.float32
    bf16 = mybir.dt.bfloat16
    f8 = mybir.dt.float8e4
    AF = mybir.ActivationFunctionType
    ALU = mybir.AluOpType
    AX = mybir.AxisListType
    DR = mybir.MatmulPerfMode.DoubleRow
    RADD = bass_isa.ReduceOp.add

    nc = bacc.Bacc("TRN2", target_bir_lowering=False, debug=False,
                   num_devices=NCORES)

    # ---- DRAM I/O ----
    xbT = nc.dram_tensor("xbT", [D, L], f32, kind="ExternalInput").ap()
    tb = nc.dram_tensor("tb", [P, 2], bf16, kind="ExternalInput").ap()
    modw = nc.dram_tensor("modw", [DT, 6 * D], bf16, kind="ExternalInput").ap()
    wq = nc.dram_tensor("wq", [D, D], f8, kind="ExternalInput").ap()
    wk = nc.dram_tensor("wk", [D, D], f8, kind="ExternalInput").ap()
    wv = nc.dram_tensor("wv", [D, D], f8, kind="ExternalInput").ap()
    wo = nc.dram_tensor("wo", [D, D], f8, kind="ExternalInput").ap()
    wg = nc.dram_tensor("wg", [D, DH], f8, kind="ExternalInput").ap()
    wh = nc.dram_tensor("wh", [D, DH], f8, kind="ExternalInput").ap()
    wout = nc.dram_tensor("wout", [DH, D], f8, kind="ExternalInput").ap()
    outbr = nc.dram_tensor("outbr", [1, D], bf16, kind="ExternalInput").ap()
    y = nc.dram_tensor("y", [D, LOWN], f32, kind="ExternalOutput").ap()

    xbT_v = xbT.rearrange("(o p) t -> p o t", p=P)      # [128, 8, 2048]
    modw_v = modw.rearrange("(c p) n -> p c n", p=P)    # [128, 2, 6144]
    wq_v = wq.rearrange("(o p) n -> p o n", p=P)        # [128, 8, 1024]
    wk_v = wk.rearrange("(o p) n -> p o n", p=P)
    wv_v = wv.rearrange("(o p) n -> p o n", p=P)
    wo_v = wo.rearrange("(o p) n -> p o n", p=P)
    wg_v = wg.rearrange("(o p) n -> p o n", p=P)        # [128, 8, 4096]
    wh_v = wh.rearrange("(o p) n -> p o n", p=P)
    wout_v = wout.rearrange("(o p) n -> p o n", p=P)    # [128, 32, 1024]
    y_v = y.rearrange("(o p) t -> p o t", p=P)          # [128, 8, 1024]

    def pair1(ap2):
        # [P, 2] AP -> [P, 2, 1] for DoubleRow N=1 rhs
        return ap2.rearrange("p (two one) -> p two one", one=1)

    with tile.TileContext(nc) as tc, ExitStack() as top:
        TPool = tc.tile_pool
        constp = top.enter_context(TPool(name="const", bufs=1))
        ones_f8 = constp.tile([P, 1], f8, name="ones_f8")
        nc.vector.memset(ones_f8[:], 1.0)
        ones8r = constp.tile([1, QC], bf16, name="ones8r")
        nc.vector.memset(ones8r[:], 1.0)
        eps_sb = constp.tile([P, 1], f32, name="eps_sb")
        nc.vector.memset(eps_sb[:], EPS)
        tb_sb = constp.tile([P, 2], bf16, name="tb_sb")
        nc.sync.dma_start(tb_sb[:], tb)
        outb_row = constp.tile([1, D], bf16, name="outb_row")
        nc.sync.dma_start(outb_row[:], outbr)
        # modulation vectors: col j*8+c is (vector j, d-chunk c); j order:
        # attn_gamma, attn_beta, attn_alpha, ffn_gamma, ffn_beta, ffn_alpha
        mods = constp.tile([P, 48], f32, name="mods")
        mods_f8 = constp.tile([P, 48], f8, name="mods_f8")
        wbeta = constp.tile([P, 24], f32, name="wbeta")     # q, k, v
        wbv_f8 = constp.tile([P, 8], f8, name="wbv_f8")
        wbo_sb = constp.tile([P, 8], f32, name="wbo_sb")
        vsum_sb = constp.tile([65, NH], f32, name="vsum_sb")

        def emit_mods(p0ps, modw_sb, ch_range, ch0):
            for ch in ch_range:  # 48 cols in groups of 4
                pc = p0ps.tile([P, 4], f32, tag="pc", name=f"pc{ch}")
                for g in range(4):
                    m = (ch - ch0) * 4 + g
                    for kc in range(2):
                        nc.tensor.matmul(
                            pc[:, g:g + 1],
                            lhsT=modw_sb[:, kc, m * P:(m + 1) * P],
                            rhs=tb_sb[:, kc:kc + 1],
                            start=(g == 0 and kc == 0),
                            stop=(g == 3 and kc == 1))
                nc.vector.tensor_copy(mods[:, ch * 4:(ch + 1) * 4], pc[:])

        # ---------- persistent attention tensors ----------
        persA = tc.alloc_tile_pool(name="persA", bufs=1)
        kT = persA.tile([P, 8, L], f8, name="kT")           # [d, hp, k-tok]
        vA = persA.tile([P, 16, NH * 65], f8, name="vA")    # v_aug per k-chunk
        qT = persA.tile([P, 8, LOWN], f8, name="qT")

        # ---------- phase 0+1: mods + attn AdaRMSNorm + QKV ----------
        with TPool(name="p0", bufs=1) as p0, \
             TPool(name="p0ps", bufs=2, space="PSUM") as p0ps, \
             TPool(name="p1w", bufs=1) as p1w, \
             TPool(name="p1x", bufs=5) as p1x, \
             TPool(name="p1s", bufs=2) as p1s, \
             TPool(name="p1r", bufs=3) as p1r, \
             TPool(name="p1n", bufs=8) as p1n, \
             TPool(name="p1ps_s", bufs=2, space="PSUM") as p1ps_s, \
             TPool(name="p1ps_a", bufs=4, space="PSUM") as p1ps_a:
            wq_sb = p1w.tile([P, 8, D], f8, name="wq_sb")
            wk_sb = p1w.tile([P, 8, D], f8, name="wk_sb")
            wv_sb = p1w.tile([P, 8, D], f8, name="wv_sb")
            modw_att = p0.tile([P, 2, 2 * D], bf16, name="modw_att")
            modw_rest = p0.tile([P, 2, 4 * D], bf16, name="modw_rest")
            wo_sb = persA.tile([P, 8, D], f8, name="wo_sb")
            xtiles = {}

            def load_x(blk):
                t = p1x.tile([P, 8, TB], f32, tag="xblk", name=f"xblk{blk}")
                nc.sync.dma_start(t[:], xbT_v[:, :, blk * TB:(blk + 1) * TB])
                xtiles[blk] = t

            # DMA priority order: x blocks race ahead of weights
            load_x(0)
            load_x(1)
            nc.sync.dma_start(modw_att[:], modw_v[:, :, 0:2 * D])
            nc.sync.dma_start(wk_sb[:], wk_v)
            emit_mods(p0ps, modw_att, range(4), 0)   # attn gamma/beta
            nc.vector.tensor_copy(mods_f8[:, 8:16], mods[:, 8:16])
            load_x(2)
            load_x(3)
            nc.sync.dma_start(wq_sb[:], wq_v)
            nc.sync.dma_start(wv_sb[:], wv_v)
            nc.sync.dma_start(wo_sb[:], wo_v)

            # ones columns of v_aug (col h*65+64 = 1)
            nc.vector.memset(
                vA.rearrange("p c (h e) -> p c h e", e=65)[:, :, :, 64:65], 1.0)

            def emit_wbeta():
                # wbeta[:, 0:8]=Wq@beta, 8:16=Wk@beta, 16:24=Wv@beta
                pwb = p0ps.tile([P, 24], f32, tag="pc", name="pwb")
                for wi, w_sb in enumerate((wq_sb, wk_sb, wv_sb)):
                    for m in range(8):
                        for j in range(4):
                            nc.tensor.matmul(
                                pwb[:, wi * 8 + m: wi * 8 + m + 1],
                                lhsT=w_sb[:, 2 * j:2 * j + 2,
                                          m * P:(m + 1) * P],
                                rhs=pair1(mods_f8[:, 8 + 2 * j:10 + 2 * j]),
                                start=(wi == 0 and m == 0 and j == 0),
                                stop=(wi == 2 and m == 7 and j == 3),
                                perf_mode=DR)
                nc.vector.tensor_copy(wbeta[:], pwb[:])
                nc.vector.tensor_copy(wbv_f8[:], pwb[:, 16:24])
                # wbo = Wo @ (Wv beta): exact V-beta correction at O-proj
                pwo = p0ps.tile([P, 8], f32, tag="pc", name="pwo")
                for m in range(8):
                    for j in range(4):
                        nc.tensor.matmul(
                            pwo[:, m:m + 1],
                            lhsT=wo_sb[:, 2 * j:2 * j + 2, m * P:(m + 1) * P],
                            rhs=pair1(wbv_f8[:, 2 * j:2 * j + 2]),
                            start=(m == 0 and j == 0),
                            stop=(m == 7 and j == 3), perf_mode=DR)
                nc.vector.tensor_copy(wbo_sb[:], pwo[:])

            emit_wbeta()

            xns = {}
            # pass A: stats + norm + K projection for every block (kT first)
            for blk in range(NBLK):
                tsl = slice(blk * TB, (blk + 1) * TB)
                xblk = xtiles.pop(blk)
                if blk + 4 < NBLK:
                    load_x(blk + 4)
                if blk == 1:
                    nc.sync.dma_start(modw_rest[:],
                                      modw_v[:, :, 2 * D:6 * D])
                    emit_mods(p0ps, modw_rest, range(4, 12), 4)
                # rms statistics: sum_d x^2 via DR ones-matmul on fp8 squares
                sq = p1s.tile([P, 8, TB], f8, tag="sq", name=f"sq{blk}")
                nc.gpsimd.tensor_tensor(sq[:], xblk[:], xblk[:], ALU.mult)
                ps_s = p1ps_s.tile([1, TB], f32, tag="ps_s", name=f"pss{blk}")
                for o in range(8):
                    nc.tensor.matmul(ps_s[:], lhsT=ones_f8[:],
                                     rhs=sq[:, o, :],
                                     start=(o == 0), stop=(o == 7))
                srow = p1r.tile([1, TB], f32, tag="srow", name=f"srow{blk}")
                nc.scalar.activation(srow[:], ps_s[:], AF.Sqrt,
                                     scale=1.0 / D, bias=eps_sb[0:1, :])
                rrow = p1r.tile([1, TB], f32, tag="rrow", name=f"rrow{blk}")
                nc.vector.reciprocal(rrow[:], srow[:])
                rbc = p1r.tile([P, TB], f32, tag="rbc", name=f"rbc{blk}")
                nc.gpsimd.partition_broadcast(rbc[:], rrow[:])
                # xn = gamma * (x * r)   (beta folded into evacuations)
                xn = p1n.tile([P, 8, TB], f8, tag="xn", name=f"xn{blk}")
                for o in range(8):
                    nc.vector.scalar_tensor_tensor(
                        xn[:, o, :], xblk[:, o, :], mods[:, o:o + 1], rbc[:],
                        op0=ALU.mult, op1=ALU.mult)
                xns[blk] = xn
                for m in range(8):
                    pp = p1ps_a.tile([P, TB], f32, tag="ppa",
                                     name=f"paK{blk}_{m}")
                    for j in range(4):
                        nc.tensor.matmul(
                            pp[:],
                            lhsT=wk_sb[:, 2 * j:2 * j + 2, m * P:(m + 1) * P],
                            rhs=xn[:, 2 * j:2 * j + 2, :],
                            start=(j == 0), stop=(j == 3), perf_mode=DR)
                    nc.scalar.activation(kT[:, m, tsl], pp[:], AF.Identity,
                                         bias=wbeta[:, 8 + m:9 + m])
            # pass B: Q projection for own blocks (evacs split ACT/DVE)
            for blk in range(LOWN // TB):
                tsl = slice(blk * TB, (blk + 1) * TB)
                xn = xns[blk]
                for m in range(8):
                    pp = p1ps_a.tile([P, TB], f32, tag="ppa",
                                     name=f"paQ{blk}_{m}")
                    for j in range(4):
                        nc.tensor.matmul(
                            pp[:],
                            lhsT=wq_sb[:, 2 * j:2 * j + 2, m * P:(m + 1) * P],
                            rhs=xn[:, 2 * j:2 * j + 2, :],
                            start=(j == 0), stop=(j == 3), perf_mode=DR)
                    if m % 2 == 0:
                        nc.vector.tensor_scalar_add(qT[:, m, tsl], pp[:],
                                                    wbeta[:, m:m + 1])
                    else:
                        nc.scalar.activation(qT[:, m, tsl], pp[:],
                                             AF.Identity,
                                             bias=wbeta[:, m:m + 1])
            # pass C: V projection (evacs split DVE/ACT)
            for blk in range(NBLK):
                xn = xns.pop(blk)
                for mt in range(TB // P):
                    kcg = blk * (TB // P) + mt  # global k-token chunk
                    for vc in range(4):
                        pv = p1ps_a.tile([P, 256], f32, tag="ppa",
                                         name=f"pv{blk}_{mt}_{vc}")
                        for j in range(4):
                            nc.tensor.matmul(
                                pv[:],
                                lhsT=xn[:, 2 * j:2 * j + 2,
                                        mt * P:(mt + 1) * P],
                                rhs=wv_sb[:, 2 * j:2 * j + 2,
                                          vc * 256:(vc + 1) * 256],
                                start=(j == 0), stop=(j == 3), perf_mode=DR)
                        dst = vA.rearrange("p c (h e) -> p c h e", e=65)[
                            :, kcg, vc * 4:(vc + 1) * 4, 0:64]
                        if vc % 2 == 0:
                            nc.vector.tensor_copy(
                                dst, pv.rearrange("p (h e) -> p h e", e=64))
                        else:
                            nc.scalar.activation(
                                dst, pv.rearrange("p (h e) -> p h e", e=64),
                                AF.Identity)
            # per-head V-sums over the Pool-quad kc range
            pvs = p1ps_a.tile([65, NH], f32, tag="ppa", name="pvs")
            QUAD_KC = (4, 5, 6, 7, 12, 13, 14, 15)
            for h in range(NH):
                for ki, kc in enumerate(QUAD_KC):
                    nc.tensor.matmul(
                        pvs[:, h:h + 1],
                        lhsT=vA[:, kc, h * 65:h * 65 + 65],
                        rhs=ones_f8[:],
                        start=(h == 0 and ki == 0),
                        stop=(h == NH - 1 and ki == 7))
            nc.vector.tensor_copy(vsum_sb[:], pvs[:])
        # ---------- persistent FFN weights (loaded during window 0) ------
        persW = tc.alloc_tile_pool(name="persW", bufs=1, side="right")
        wg_sb = persW.tile([P, 8, DH], f8, name="wg_sb")
        wout_sb = persW.tile([P, 32, D], f8, name="wout_sb")
        for hb in range(4):
            hsl = slice(hb * D, (hb + 1) * D)
            nc.sync.dma_start(wg_sb[:, :, hsl], wg_v[:, :, hsl])
        for ob in range(4):
            nc.sync.dma_start(wout_sb[:, 8 * ob:8 * ob + 8, :],
                              wout_v[:, 8 * ob:8 * ob + 8, :])

        # ---------- chunk-pipelined attention + FFN ----------
        chp = tc.alloc_tile_pool(name="chp", bufs=1)
        oT = [chp.tile([P, 8, QC], f8, name=f"oT{i}") for i in range(2)]

        p_sc = tc.alloc_tile_pool(name="p_sc", bufs=2, space="PSUM")
        p_av = tc.alloc_tile_pool(name="p_av", bufs=1, space="PSUM")
        p_e = tc.alloc_tile_pool(name="p_e", bufs=6)
        p_q = tc.alloc_tile_pool(name="p_q", bufs=2) if pool_set else None
        p_r = tc.alloc_tile_pool(name="p_r", bufs=4)

        avT = [None]
        etiles = {}

        def emit_score_group(c, h, g, qsl):
            hp, hh = h // 2, h % 2
            rows = slice(64 * hh, 64 * hh + 64)
            S = p_sc.tile([P, 4, QC], f32, tag="sc", name=f"S{c}_{h}_{g}")
            for i in range(4):
                kc = 4 * g + i
                nc.tensor.matmul(
                    S[:, i, :],
                    lhsT=kT[rows, hp, kc * P:(kc + 1) * P],
                    rhs=qT[rows, hp, qsl],
                    start=(i % 2 == 0), stop=(i % 2 == 1))
            et = p_e.tile([P, 4, QC], f8, tag="e", name=f"e{c}_{h}_{g}")
            if h in pool_set and g % 2 == 1:
                # e' = s'(1 + s'/2), s' = SM*s  (exact +1 via vsum)
                p1t = p_q.tile([P, 4, QC], f8, tag="p1",
                               name=f"p1{c}_{h}_{g}")
                nc.gpsimd.tensor_scalar(p1t[:], S[:], 0.5 * SM * SM, SM,
                                        op0=ALU.mult, op1=ALU.add)
                nc.gpsimd.scalar_tensor_tensor(
                    et[:], p1t[:], 1.0, S[:], op0=ALU.mult, op1=ALU.mult)
            else:
                nc.scalar.activation(et[:], S[:], AF.Exp, scale=SM)
            etiles[(h, g)] = et

        def emit_av_group(c, h, g):
            if h % 2 == 0 and g == 0:
                avT[0] = p_av.tile([P, 2, QC], f32, tag="av",
                                   name=f"av{c}_{h}")
            po = avT[0][0:65, h % 2, :]
            et = etiles.pop((h, g))
            for jj in range(2):
                kp = 2 * g + jj
                nc.tensor.matmul(
                    po,
                    lhsT=vA[:, 2 * kp:2 * kp + 2, h * 65:h * 65 + 65],
                    rhs=et[:, 2 * jj:2 * jj + 2, :],
                    start=(kp == 0 and h % 2 == 0),
                    stop=(kp == 7 and h % 2 == 1), perf_mode=DR)

        def emit_head_evac(c, h):
            hp, hh = h // 2, h % 2
            po = avT[0][0:65, h % 2, :]
            rec = p_r.tile([1, QC], f32, tag="rec", name=f"rec{c}_{h}")
            if h in pool_set:
                den = p_r.tile([1, QC], f32, tag="den", name=f"den{c}_{h}")
                nc.vector.tensor_scalar_add(den[:], po[64:65, :], 1024.0)
                nc.vector.reciprocal(rec[:], den[:])
            else:
                nc.vector.reciprocal(rec[:], po[64:65, :])
            bc = p_r.tile([64, QC], f32, tag="bc", name=f"bc{c}_{h}")
            nc.gpsimd.partition_broadcast(bc[:], rec[:])
            dst = oT[c % 2][64 * hh:64 * hh + 64, hp, :]
            if h in pool_set:
                nc.vector.scalar_tensor_tensor(
                    dst, po[0:64, :], vsum_sb[0:64, h:h + 1], bc[:],
                    op0=ALU.add, op1=ALU.mult)
            else:
                nc.vector.tensor_tensor(dst, po[0:64, :], bc[:], ALU.mult)

        # ---- FFN for chunk u, emitted as fine-grained quanta ----
        def gen_oproj(c, qsl):
            """O-projection + residual + ffn-stats partials for chunk c."""
            ot = oT[c % 2]
            xr = p_xr.tile([P, 8, QC], f32, tag="xr", name=f"xr{c}")
            nc.sync.dma_start(xr[:], xbT_v[:, :, qsl])
            red = p_st.tile([P, QC], f32, tag="red", name=f"red{c}")
            ctx_red[0] = red
            for dp in range(4):  # dm-pairs
                pO = pools["dn"].tile([P, 2, QC], f32, tag="dn",
                                      name=f"O{c}_{dp}")
                for i in range(2):
                    dm = 2 * dp + i
                    for j in range(4):
                        nc.tensor.matmul(
                            pO[:, i, :],
                            lhsT=wo_sb[:, 2 * j:2 * j + 2,
                                       dm * P:(dm + 1) * P],
                            rhs=ot[:, 2 * j:2 * j + 2, :],
                            start=(i == 0 and j == 0),
                            stop=(i == 1 and j == 3), perf_mode=DR)
                    yield 'pe'
                for i in range(2):
                    dm = 2 * dp + i
                    t1 = p_yt.tile([P, QC], f32, tag="yt",
                                   name=f"t1{c}_{dp}_{i}")
                    nc.vector.tensor_scalar_add(t1[:], pO[:, i, :],
                                                wbo_sb[:, dm:dm + 1])
                    # x2 = x + a_alpha * o_proj
                    nc.vector.scalar_tensor_tensor(
                        x2[:, dm, :], t1[:], mods[:, 16 + dm:17 + dm],
                        xr[:, dm, :], op0=ALU.mult, op1=ALU.add)
                # ffn-stats partial for this dm-pair
                sqp = p_st.tile([P, 2, QC], bf16, tag="sqp",
                                name=f"sqp{c}_{dp}")
                nc.vector.tensor_tensor(sqp[:], x2[:, 2 * dp:2 * dp + 2, :],
                                        x2[:, 2 * dp:2 * dp + 2, :], ALU.mult)
                redp = p_st.tile([P, QC], f32, tag="redp",
                                 name=f"redp{c}_{dp}")
                nc.vector.tensor_reduce(redp[:],
                                        sqp.rearrange("p o t -> p t o"),
                                        AX.X, ALU.add)
                if dp == 0:
                    nc.vector.tensor_copy(red[:], redp[:])
                else:
                    nc.vector.tensor_tensor(red[:], red[:], redp[:], ALU.add)
                yield 'lite'

        def gen_ffnnorm(c):
            """ffn AdaRMSNorm for chunk c (rsqrt via DVE Newton)."""
            red = ctx_red[0]
            mred = p_st.tile([P, QC], f32, tag="mred", name=f"mred{c}")
            nc.gpsimd.partition_all_reduce(mred[:], red[:], channels=P,
                                           reduce_op=RADD)
            # 2 Newton iterations for rsqrt(mred/D + eps), y0 = 1
            y1 = p_r.tile([P, QC], f32, tag="y1", name=f"y1{c}")
            nc.vector.tensor_scalar(y1[:], mred[:], -0.5 / D,
                                    1.5 - 0.5 * EPS,
                                    op0=ALU.mult, op1=ALU.add)
            ya = p_r.tile([P, QC], f32, tag="ya", name=f"ya{c}")
            nc.vector.tensor_tensor(ya[:], y1[:], y1[:], ALU.mult)
            yb = p_r.tile([P, QC], f32, tag="yb", name=f"yb{c}")
            nc.vector.scalar_tensor_tensor(yb[:], mred[:], -0.5 / D, ya[:],
                                           op0=ALU.mult, op1=ALU.mult)
            nc.vector.tensor_scalar_add(yb[:], yb[:], 1.5)
            rbc2 = p_r.tile([P, QC], f32, tag="rbc2", name=f"rbc2{c}")
            nc.vector.tensor_tensor(rbc2[:], y1[:], yb[:], ALU.mult)
            yield 'lite'
            for o in range(8):
                nc.vector.scalar_tensor_tensor(
                    xn2[:, o, :], x2[:, o, :], mods[:, 24 + o:25 + o],
                    rbc2[:], op0=ALU.mult, op1=ALU.mult)
                nc.vector.tensor_scalar_add(xn2[:, o, :], xn2[:, o, :],
                                            mods[:, 32 + o:33 + o])
                if o == 3:
                    yield 'lite'
            yield 'lite'

        def gen_up_pair(c, pr, wh_t):
            """SwiGLU up + silu-via-exp for mi pair (2pr, 2pr+1)."""
            upT = pools["up"].tile([P, 4, QC], f32, tag="up",
                                   name=f"up{c}_{pr}")
            for w_sb, base in ((wg_sb, 0), (wh_t, 2)):
                for i in range(2):
                    mi = 2 * pr + i
                    wsl = (slice(mi * P, (mi + 1) * P) if w_sb is wg_sb
                           else slice(i * P, (i + 1) * P))
                    for j in range(4):
                        nc.tensor.matmul(
                            upT[:, base + i, :],
                            lhsT=w_sb[:, 2 * j:2 * j + 2, wsl],
                            rhs=xn2[:, 2 * j:2 * j + 2, :],
                            start=(i == 0 and j == 0),
                            stop=(i == 1 and j == 3), perf_mode=DR)
                    yield 'pe'
            # silu(g)*h = g*h / (1 + e^-g)
            # evacuate psum first (frees the up tile for the next pair)
            gha = p_sg.tile([P, 4, QC], bf16, tag="gha", name=f"gha{c}_{pr}")
            nc.vector.tensor_copy(gha[:], upT[:])
            yield 'lite'
            eg = p_sg.tile([P, 2, QC], bf16, tag="eg", name=f"eg{c}_{pr}")
            nc.scalar.activation(eg[:], gha[:, 0:2, :], AF.Exp, scale=-1.0)
            nc.vector.tensor_scalar_add(eg[:], eg[:], 1.0)
            with nc.allow_low_precision(reason="silu sigmoid in bf16"):
                nc.vector.reciprocal(eg[:], eg[:])
            # m = (g * sigmoid(g)) * h, all operands in SBUF
            u = p_sg.tile([P, 2, QC], bf16, tag="gh", name=f"u{c}_{pr}")
            nc.vector.tensor_tensor(u[:], gha[:, 0:2, :], eg[:], ALU.mult)
            nc.gpsimd.tensor_tensor(m_sb[:, 2 * pr:2 * pr + 2, :],
                                    gha[:, 2:4, :], u[:], ALU.mult)
            yield 'lite'

        def gen_down(c, dp, qsl):
            """down-proj for dm pair (2dp, 2dp+1) of chunk c + y out."""
            pD = pools["dn"].tile([P, 2, QC], f32, tag="dn",
                                  name=f"D{c}_{dp}")
            for i in range(2):
                dm = 2 * dp + i
                for mp in range(16):
                    nc.tensor.matmul(
                        pD[:, i, :],
                        lhsT=wout_sb[:, 2 * mp:2 * mp + 2,
                                     dm * P:(dm + 1) * P],
                        rhs=m_sb[:, 2 * mp:2 * mp + 2, :],
                        start=(i == 0 and mp == 0), stop=False, perf_mode=DR)
                    if mp % 4 == 3:
                        yield 'pe'
                nc.tensor.matmul(
                    pD[:, i, :], lhsT=outb_row[0:1, dm * P:(dm + 1) * P],
                    rhs=ones8r[:], start=False, stop=(i == 1))
                yield 'pe'
            for i in range(2):
                dm = 2 * dp + i
                yt = p_yt.tile([P, QC], f32, tag="yt", name=f"yt{c}_{dp}_{i}")
                nc.vector.scalar_tensor_tensor(
                    yt[:], pD[:, i, :], mods[:, 40 + dm:41 + dm],
                    x2[:, dm, :], op0=ALU.mult, op1=ALU.add)
                nc.sync.dma_start(y_v[:, dm, qsl], yt[:])
            yield 'lite'

        ctx_red = [None]
        from collections import deque
        gens = deque()

        def pull(budget=1):
            lite_run = 0
            while gens and budget > 0:
                try:
                    tag = next(gens[0])
                except StopIteration:
                    gens.popleft()
                    continue
                if tag == 'pe':
                    budget -= 1
                else:
                    lite_run += 1
                    if lite_run >= 3:
                        budget -= 1

        def emit_attn_window(a, qsl_a):
            for h in range(NH):
                for g in range(4):
                    if h >= 1:
                        emit_av_group(a, h - 1, g)
                    pull()
                    emit_score_group(a, h, g, qsl_a)
                    pull()
                if h >= 2 and h % 2 == 0:
                    emit_head_evac(a, h - 2)
                    emit_head_evac(a, h - 1)
            for g in range(4):
                emit_av_group(a, NH - 1, g)
                pull()
            emit_head_evac(a, NH - 2)
            emit_head_evac(a, NH - 1)

        # ---- window 0: chunk-0 attention ----
        emit_attn_window(0, slice(0, QC))

        x2 = chp.tile([P, 8, QC], bf16, name="x2")
        xn2 = chp.tile([P, 8, QC], f8, name="xn2")
        m_sb = chp.tile([P, 32, QC], f8, name="m_sb")
        p_sg = tc.alloc_tile_pool(name="p_sg", bufs=2)
        p_yt = tc.alloc_tile_pool(name="p_yt", bufs=3)
        p_st = tc.alloc_tile_pool(name="p_st", bufs=1)
        p_wh = tc.alloc_tile_pool(name="p_wh", bufs=2)
        p_xr = tc.alloc_tile_pool(name="p_xr", bufs=2)
        pools = {"up": tc.alloc_tile_pool(name="p_up", bufs=1, space="PSUM"),
                 "dn": tc.alloc_tile_pool(name="p_dn", bufs=1, space="PSUM"),
                 "pool_mult": False}

        for w in range(1, NQC + 1):
            a, u = w, w - 1  # attention chunk / FFN chunk this window
            qsl_a = slice((a % NQC) * QC, (a % NQC + 1) * QC)
            qsl_u = slice(u * QC, (u + 1) * QC)
            wh_tiles = []
            for pr in range(16):
                wt = p_wh.tile([P, 8, 2 * P], f8, tag="wh",
                               name=f"wh{w}_{pr}")
                nc.sync.dma_start(
                    wt[:], wh_v[:, :, 2 * pr * P:(2 * pr + 2) * P])
                wh_tiles.append(wt)
            if a == NQC:
                # drain: rebuild up/dn pools with double buffering
                pools["dn"].release()
                pools["up"].release()
                p_av.release()
                p_sc.release()
                pools["up"] = tc.alloc_tile_pool(name="p_up2", bufs=3,
                                                 space="PSUM")
                pools["dn"] = tc.alloc_tile_pool(name="p_dn2", bufs=2,
                                                 space="PSUM")
                pools["pool_mult"] = True
            gens.append(gen_oproj(u, qsl_u))
            gens.append(gen_ffnnorm(u))
            for pr in range(16):
                gens.append(gen_up_pair(u, pr, wh_tiles[pr]))
            for dp in range(4):
                gens.append(gen_down(u, dp, qsl_u))
            if a < NQC:
                emit_attn_window(a, qsl_a)
                while gens:
                    pull(budget=100)
            else:
                # drain window: free attention psum, double-buffer the ups
                while gens:
                    pull(budget=100)

        pools["dn"].release()
        pools["up"].release()
        for pool in (p_xr, p_wh, p_st, p_yt, p_sg, p_r, p_e,
                     chp, persW, persA):
            pool.release()
        if p_q is not None:
            p_q.release()

    nc.compile()
    return nc


def _get_nc():
    if "nc" not in _CACHE:
        _CACHE["nc"] = _build_nc()
    return _CACHE["nc"]


def make_in_maps(x, t, attn_gamma_w, attn_beta_w, W_q, W_k, W_v, W_o,
                 attn_alpha_w, ffn_gamma_w, ffn_beta_w, gate_w, hidden_w,
                 out_w, out_b, ffn_alpha_w):
    import ml_dtypes
    bf = ml_dtypes.bfloat16
    f8 = ml_dtypes.float8_e4m3
    f32 = np.float32

    def T8(a):
        return np.ascontiguousarray(np.asarray(a, f32).T).astype(f8)

    xT = np.ascontiguousarray(np.asarray(x, f32).transpose(0, 2, 1))
    t = np.asarray(t, f32)
    modw = np.ascontiguousarray(np.concatenate(
        [np.asarray(w, f32) for w in (attn_gamma_w, attn_beta_w, attn_alpha_w,
                                      ffn_gamma_w, ffn_beta_w, ffn_alpha_w)],
        axis=0).T).astype(bf)                          # [256, 6144]
    shared = {
        "modw": modw,
        "wq": T8(W_q), "wk": T8(W_k), "wv": T8(W_v), "wo": T8(W_o),
        "wg": T8(gate_w), "wh": T8(hidden_w), "wout": T8(out_w),
        "outbr": np.ascontiguousarray(
            np.asarray(out_b, f32).reshape(1, D)).astype(bf),
    }
    in_maps = []
    for c in range(NCORES):
        b, h = c // 2, c % 2
        if h == 0:
            xbT = xT[b]
        else:
            xbT = np.concatenate([xT[b][:, LOWN:], xT[b][:, :LOWN]], axis=1)
        in_maps.append(dict(
            shared,
            xbT=np.ascontiguousarray(xbT),
            tb=np.ascontiguousarray(t[b].reshape(2, P).T).astype(bf),
        ))
    return in_maps


def kernel(**inputs):
    from concourse.bass_utils import run_bass_kernel_spmd

    nc = _get_nc()
    in_maps = make_in_maps(**inputs)
    res = run_bass_kernel_spmd(nc, in_maps, core_ids=list(range(NCORES)))
    x = np.asarray(inputs["x"])
    yfull = np.empty((x.shape[0], L, D), dtype=np.float32)
    for c in range(NCORES):
        b, h = c // 2, c % 2
        yfull[b, h * LOWN:(h + 1) * LOWN, :] = res.results[c]["y"].T
    return yfull


# revision 5
# speedup vs baseline: 1.6100x; 1.0309x over previous
"""Trainium2 Bass kernel for a DiT block (AdaRMSNorm + MHA + AdaRMSNorm + SwiGLU).

Sharding: 8 cores = 4 batches x 2 query-halves (as the baseline).  Each core
owns 1024 query tokens of one batch; K/V over the full 2048 tokens of its
batch; zero collectives.

Speed strategy vs the bf16 baseline:
  - All projection/AV GEMMs run in fp8e4 with DoubleRow perf mode (2 k-tiles
    contracted per instruction at 0.5 cycles/row).  Scores run in plain fp8.
  - The block is pipelined over 4 query-chunks of 256: window w runs
    attention (scores -> exp -> AV -> oT) for chunk w interleaved with the
    complete FFN of chunk w-1, keeping PE/DVE/Pool busy under the
    Activation-engine exp stream.
  - exp for a subset of heads runs on the Pool engine as a quadratic
    e' = s'(1 + s'/2) (logits are within [-0.5, 0.5]); the missing "+1" is
    recovered exactly through per-head V-sum columns and a +2048 denominator
    shift, since softmax weights sum to one.
  - silu is computed via Exp (same ACT table as attention exp, so the ACT
    engine never reloads activation tables): silu(g)*h = g*h/(1+e^-g).
  - AdaRMSNorm betas for the attention branch are folded into the PSUM
    evacuations (K/Q via ACT bias; V exactly via Wo@(Wv beta) at the
    O-projection).  ffn-norm rsqrt runs on DVE with 2 Newton iterations.
  - wh streams per window; wg/wout/wo stay resident (SBUF budget).
"""

import numpy as np

P = 128
D = 1024
DT = 256
DH = 4096
NH = 16
L = 2048
LOWN = 1024
QC = 256
NQC = 4
TB = 256
NBLK = 8
EPS = 1e-6
SM = 0.125  # 1/sqrt(d_head)
NCORES = 8

_CACHE = {}


def _build_nc():
    from contextlib import ExitStack
    import os
    _POOL_HEADS = int(os.environ.get("KERNEL_POOL_HEADS", "0"))
    # heads whose score-groups 1 and 3 take the Pool-quad softmax path
    pool_set = set(range(1, 2 * min(_POOL_HEADS, 16), 2))[:_POOL_HEADS] \
        if False else set(list(range(1, 16, 2))[:_POOL_HEADS])

    import concourse.bass as bass  # noqa: F401
    import concourse.tile as tile
    from concourse import bacc, mybir
    from concourse import bass_isa

    f32 = mybir.dt.float32
    bf16 = mybir.dt.bfloat16
    f8 = mybir.dt.float8e4
    AF = mybir.ActivationFunctionType
    ALU = mybir.AluOpType
    AX = mybir.AxisListType
    DR = mybir.MatmulPerfMode.DoubleRow
    RADD = bass_isa.ReduceOp.add

    nc = bacc.Bacc("TRN2", target_bir_lowering=False, debug=False,
                   num_devices=NCORES)

    # ---- DRAM I/O ----
    xbT = nc.dram_tensor("xbT", [D, L], f32, kind="ExternalInput").ap()
    tb = nc.dram_tensor("tb", [P, 2], bf16, kind="ExternalInput").ap()
    modw = nc.dram_tensor("modw", [DT, 6 * D], bf16, kind="ExternalInput").ap()
    wq = nc.dram_tensor("wq", [D, D], f8, kind="ExternalInput").ap()
    wk = nc.dram_tensor("wk", [D, D], f8, kind="ExternalInput").ap()
    wv = nc.dram_tensor("wv", [D, D], f8, kind="ExternalInput").ap()
    wo = nc.dram_tensor("wo", [D, D], f8, kind="ExternalInput").ap()
    wg = nc.dram_tensor("wg", [D, DH], f8, kind="ExternalInput").ap()
    wh = nc.dram_tensor("wh", [D, DH], f8, kind="ExternalInput").ap()
    wout = nc.dram_tensor("wout", [DH, D], f8, kind="ExternalInput").ap()
    outbr = nc.dram_tensor("outbr", [1, D], bf16, kind="ExternalInput").ap()
    y = nc.dram_tensor("y", [D, LOWN], f32, kind="ExternalOutput").ap()

    xbT_v = xbT.rearrange("(o p) t -> p o t", p=P)      # [128, 8, 2048]
    modw_v = modw.rearrange("(c p) n -> p c n", p=P)    # [128, 2, 6144]
    wq_v = wq.rearrange("(o p) n -> p o n", p=P)        # [128, 8, 1024]
    wk_v = wk.rearrange("(o p) n -> p o n", p=P)
    wv_v = wv.rearrange("(o p) n -> p o n", p=P)
    wo_v = wo.rearrange("(o p) n -> p o n", p=P)
    wg_v = wg.rearrange("(o p) n -> p o n", p=P)        # [128, 8, 4096]
    wh_v = wh.rearrange("(o p) n -> p o n", p=P)
    wout_v = wout.rearrange("(o p) n -> p o n", p=P)    # [128, 32, 1024]
    y_v = y.rearrange("(o p) t -> p o t", p=P)          # [128, 8, 1024]

    def pair1(ap2):
        # [P, 2] AP -> [P, 2, 1] for DoubleRow N=1 rhs
        return ap2.rearrange("p (two one) -> p two one", one=1)

    with tile.TileContext(nc) as tc, ExitStack() as top:
        TPool = tc.tile_pool
        constp = top.enter_context(TPool(name="const", bufs=1))
        ones_f8 = constp.tile([P, 1], f8, name="ones_f8")
        nc.vector.memset(ones_f8[:], 1.0)
        ones8r = constp.tile([1, QC], bf16, name="ones8r")
        nc.vector.memset(ones8r[:], 1.0)
        eps_sb = constp.tile([P, 1], f32, name="eps_sb")
        nc.vector.memset(eps_sb[:], EPS)
        tb_sb = constp.tile([P, 2], bf16, name="tb_sb")
        nc.sync.dma_start(tb_sb[:], tb)
        outb_row = constp.tile([1, D], bf16, name="outb_row")
        nc.sync.dma_start(outb_row[:], outbr)
        # modulation vectors: col j*8+c is (vector j, d-chunk c); j order:
        # attn_gamma, attn_beta, attn_alpha, ffn_gamma, ffn_beta, ffn_alpha
        mods = constp.tile([P, 48], f32, name="mods")
        mods_f8 = constp.tile([P, 48], f8, name="mods_f8")
        wbeta = constp.tile([P, 24], f32, name="wbeta")     # q, k, v
        wbv_f8 = constp.tile([P, 8], f8, name="wbv_f8")
        wbo_sb = constp.tile([P, 8], f32, name="wbo_sb")
        vsum_sb = constp.tile([65, NH], f32, name="vsum_sb")

        def emit_mods(p0ps, modw_sb, ch_range, ch0):
            for ch in ch_range:  # 48 cols in groups of 4
                pc = p0ps.tile([P, 4], f32, tag="pc", name=f"pc{ch}")
                for g in range(4):
                    m = (ch - ch0) * 4 + g
                    for kc in range(2):
                        nc.tensor.matmul(
                            pc[:, g:g + 1],
                            lhsT=modw_sb[:, kc, m * P:(m + 1) * P],
                            rhs=tb_sb[:, kc:kc + 1],
                            start=(g == 0 and kc == 0),
                            stop=(g == 3 and kc == 1))
                nc.vector.tensor_copy(mods[:, ch * 4:(ch + 1) * 4], pc[:])

        # ---------- persistent attention tensors ----------
        persA = tc.alloc_tile_pool(name="persA", bufs=1)
        kT = persA.tile([P, 8, L], f8, name="kT")           # [d, hp, k-tok]
        vA = persA.tile([P, 16, NH * 65], f8, name="vA")    # v_aug per k-chunk
        qT = persA.tile([P, 8, LOWN], f8, name="qT")

        # ---------- phase 0+1: mods + attn AdaRMSNorm + QKV ----------
        with TPool(name="p0", bufs=1) as p0, \
             TPool(name="p0ps", bufs=2, space="PSUM") as p0ps, \
             TPool(name="p1w", bufs=1) as p1w, \
             TPool(name="p1x", bufs=5) as p1x, \
             TPool(name="p1s", bufs=2) as p1s, \
             TPool(name="p1r", bufs=3) as p1r, \
             TPool(name="p1n", bufs=8) as p1n, \
             TPool(name="p1ps_s", bufs=2, space="PSUM") as p1ps_s, \
             TPool(name="p1ps_a", bufs=4, space="PSUM") as p1ps_a:
            wq_sb = p1w.tile([P, 8, D], f8, name="wq_sb")
            wk_sb = p1w.tile([P, 8, D], f8, name="wk_sb")
            wv_sb = p1w.tile([P, 8, D], f8, name="wv_sb")
            modw_att = p0.tile([P, 2, 2 * D], bf16, name="modw_att")
            modw_rest = p0.tile([P, 2, 4 * D], bf16, name="modw_rest")
            wo_sb = persA.tile([P, 8, D], f8, name="wo_sb")
            xtiles = {}

            def load_x(blk):
                t = p1x.tile([P, 8, TB], f32, tag="xblk", name=f"xblk{blk}")
                nc.sync.dma_start(t[:], xbT_v[:, :, blk * TB:(blk + 1) * TB])
                xtiles[blk] = t

            # DMA priority order: x blocks race ahead of weights
            load_x(0)
            load_x(1)
            nc.sync.dma_start(modw_att[:], modw_v[:, :, 0:2 * D])
            nc.sync.dma_start(wk_sb[:], wk_v)
            emit_mods(p0ps, modw_att, range(4), 0)   # attn gamma/beta
            nc.vector.tensor_copy(mods_f8[:, 8:16], mods[:, 8:16])
            load_x(2)
            load_x(3)
            nc.sync.dma_start(wq_sb[:], wq_v)
            nc.sync.dma_start(wv_sb[:], wv_v)
            nc.sync.dma_start(wo_sb[:], wo_v)

            # ones columns of v_aug (col h*65+64 = 1)
            nc.vector.memset(
                vA.rearrange("p c (h e) -> p c h e", e=65)[:, :, :, 64:65], 1.0)

            def emit_wbeta():
                # wbeta[:, 0:8]=Wq@beta, 8:16=Wk@beta, 16:24=Wv@beta
                pwb = p0ps.tile([P, 24], f32, tag="pc", name="pwb")
                for wi, w_sb in enumerate((wq_sb, wk_sb, wv_sb)):
                    for m in range(8):
                        for j in range(4):
                            nc.tensor.matmul(
                                pwb[:, wi * 8 + m: wi * 8 + m + 1],
                                lhsT=w_sb[:, 2 * j:2 * j + 2,
                                          m * P:(m + 1) * P],
                                rhs=pair1(mods_f8[:, 8 + 2 * j:10 + 2 * j]),
                                start=(wi == 0 and m == 0 and j == 0),
                                stop=(wi == 2 and m == 7 and j == 3),
                                perf_mode=DR)
                nc.vector.tensor_copy(wbeta[:], pwb[:])
                nc.vector.tensor_copy(wbv_f8[:], pwb[:, 16:24])
                # wbo = Wo @ (Wv beta): exact V-beta correction at O-proj
                pwo = p0ps.tile([P, 8], f32, tag="pc", name="pwo")
                for m in range(8):
                    for j in range(4):
                        nc.tensor.matmul(
                            pwo[:, m:m + 1],
                            lhsT=wo_sb[:, 2 * j:2 * j + 2, m * P:(m + 1) * P],
                            rhs=pair1(wbv_f8[:, 2 * j:2 * j + 2]),
                            start=(m == 0 and j == 0),
                            stop=(m == 7 and j == 3), perf_mode=DR)
                nc.vector.tensor_copy(wbo_sb[:], pwo[:])

            emit_wbeta()

            xns = {}
            # pass A: stats + norm + K projection for every block (kT first)
            for blk in range(NBLK):
                tsl = slice(blk * TB, (blk + 1) * TB)
                xblk = xtiles.pop(blk)
                if blk + 4 < NBLK:
                    load_x(blk + 4)
                if blk == 1:
                    nc.sync.dma_start(modw_rest[:],
                                      modw_v[:, :, 2 * D:6 * D])
                    emit_mods(p0ps, modw_rest, range(4, 12), 4)
                # rms statistics: sum_d x^2 via DR ones-matmul on fp8 squares
                sq = p1s.tile([P, 8, TB], f8, tag="sq", name=f"sq{blk}")
                nc.gpsimd.tensor_tensor(sq[:], xblk[:], xblk[:], ALU.mult)
                ps_s = p1ps_s.tile([1, TB], f32, tag="ps_s", name=f"pss{blk}")
                for o in range(8):
                    nc.tensor.matmul(ps_s[:], lhsT=ones_f8[:],
                                     rhs=sq[:, o, :],
                                     start=(o == 0), stop=(o == 7))
                srow = p1r.tile([1, TB], f32, tag="srow", name=f"srow{blk}")
                nc.scalar.activation(srow[:], ps_s[:], AF.Sqrt,
                                     scale=1.0 / D, bias=eps_sb[0:1, :])
                rrow = p1r.tile([1, TB], f32, tag="rrow", name=f"rrow{blk}")
                nc.vector.reciprocal(rrow[:], srow[:])
                rbc = p1r.tile([P, TB], f32, tag="rbc", name=f"rbc{blk}")
                nc.gpsimd.partition_broadcast(rbc[:], rrow[:])
                # xn = gamma * (x * r)   (beta folded into evacuations)
                xn = p1n.tile([P, 8, TB], f8, tag="xn", name=f"xn{blk}")
                for o in range(8):
                    nc.vector.scalar_tensor_tensor(
                        xn[:, o, :], xblk[:, o, :], mods[:, o:o + 1], rbc[:],
                        op0=ALU.mult, op1=ALU.mult)
                xns[blk] = xn
                for m in range(8):
                    pp = p1ps_a.tile([P, TB], f32, tag="ppa",
                                     name=f"paK{blk}_{m}")
                    for j in range(4):
                        nc.tensor.matmul(
                            pp[:],
                            lhsT=wk_sb[:, 2 * j:2 * j + 2, m * P:(m + 1) * P],
                            rhs=xn[:, 2 * j:2 * j + 2, :],
                            start=(j == 0), stop=(j == 3), perf_mode=DR)
                    nc.scalar.activation(kT[:, m, tsl], pp[:], AF.Identity,
                                         bias=wbeta[:, 8 + m:9 + m])
            # pass B: Q projection for own blocks (evacs split ACT/DVE)
            for blk in range(LOWN // TB):
                tsl = slice(blk * TB, (blk + 1) * TB)
                xn = xns[blk]
                for m in range(8):
                    pp = p1ps_a.tile([P, TB], f32, tag="ppa",
                                     name=f"paQ{blk}_{m}")
                    for j in range(4):
                        nc.tensor.matmul(
                            pp[:],
                            lhsT=wq_sb[:, 2 * j:2 * j + 2, m * P:(m + 1) * P],
                            rhs=xn[:, 2 * j:2 * j + 2, :],
                            start=(j == 0), stop=(j == 3), perf_mode=DR)
                    if m % 2 == 0:
                        nc.vector.tensor_scalar_add(qT[:, m, tsl], pp[:],
                                                    wbeta[:, m:m + 1])
                    else:
                        nc.scalar.activation(qT[:, m, tsl], pp[:],
                                             AF.Identity,
                                             bias=wbeta[:, m:m + 1])
            # pass C: V projection (evacs split DVE/ACT)
            for blk in range(NBLK):
                xn = xns.pop(blk)
                for mt in range(TB // P):
                    kcg = blk * (TB // P) + mt  # global k-token chunk
                    for vc in range(4):
                        pv = p1ps_a.tile([P, 256], f32, tag="ppa",
                                         name=f"pv{blk}_{mt}_{vc}")
                        for j in range(4):
                            nc.tensor.matmul(
                                pv[:],
                                lhsT=xn[:, 2 * j:2 * j + 2,
                                        mt * P:(mt + 1) * P],
                                rhs=wv_sb[:, 2 * j:2 * j + 2,
                                          vc * 256:(vc + 1) * 256],
                                start=(j == 0), stop=(j == 3), perf_mode=DR)
                        dst = vA.rearrange("p c (h e) -> p c h e", e=65)[
                            :, kcg, vc * 4:(vc + 1) * 4, 0:64]
                        if vc % 2 == 0:
                            nc.vector.tensor_copy(
                                dst, pv.rearrange("p (h e) -> p h e", e=64))
                        else:
                            nc.scalar.activation(
                                dst, pv.rearrange("p (h e) -> p h e", e=64),
                                AF.Identity)
            # per-head V-sums over the Pool-quad kc range
            pvs = p1ps_a.tile([65, NH], f32, tag="ppa", name="pvs")
            QUAD_KC = (4, 5, 6, 7, 12, 13, 14, 15)
            for h in range(NH):
                for ki, kc in enumerate(QUAD_KC):
                    nc.tensor.matmul(
                        pvs[:, h:h + 1],
                        lhsT=vA[:, kc, h * 65:h * 65 + 65],
                        rhs=ones_f8[:],
                        start=(h == 0 and ki == 0),
                        stop=(h == NH - 1 and ki == 7))
            nc.vector.tensor_copy(vsum_sb[:], pvs[:])
        # ---------- persistent FFN weights (loaded during window 0) ------
        persW = tc.alloc_tile_pool(name="persW", bufs=1, side="right")
        wg_sb = persW.tile([P, 8, DH], f8, name="wg_sb")
        wout_sb = persW.tile([P, 32, D], f8, name="wout_sb")
        for hb in range(4):
            hsl = slice(hb * D, (hb + 1) * D)
            nc.sync.dma_start(wg_sb[:, :, hsl], wg_v[:, :, hsl])
        for ob in range(4):
            nc.sync.dma_start(wout_sb[:, 8 * ob:8 * ob + 8, :],
                              wout_v[:, 8 * ob:8 * ob + 8, :])

        # ---------- chunk-pipelined attention + FFN ----------
        chp = tc.alloc_tile_pool(name="chp", bufs=1)
        oT = [chp.tile([P, 8, QC], f8, name=f"oT{i}") for i in range(2)]

        p_sc = tc.alloc_tile_pool(name="p_sc", bufs=2, space="PSUM")
        p_av = tc.alloc_tile_pool(name="p_av", bufs=1, space="PSUM")
        p_e = tc.alloc_tile_pool(name="p_e", bufs=6)
        p_q = tc.alloc_tile_pool(name="p_q", bufs=2) if pool_set else None
        p_r = tc.alloc_tile_pool(name="p_r", bufs=4)

        avT = [None]
        etiles = {}

        def emit_score_group(c, h, g, qsl):
            hp, hh = h // 2, h % 2
            rows = slice(64 * hh, 64 * hh + 64)
            S = p_sc.tile([P, 4, QC], f32, tag="sc", name=f"S{c}_{h}_{g}")
            for i in range(4):
                kc = 4 * g + i
                nc.tensor.matmul(
                    S[:, i, :],
                    lhsT=kT[rows, hp, kc * P:(kc + 1) * P],
                    rhs=qT[rows, hp, qsl],
                    start=(i % 2 == 0), stop=(i % 2 == 1))
            et = p_e.tile([P, 4, QC], f8, tag="e", name=f"e{c}_{h}_{g}")
            if h in pool_set and g % 2 == 1:
                # e' = s'(1 + s'/2), s' = SM*s  (exact +1 via vsum)
                p1t = p_q.tile([P, 4, QC], f8, tag="p1",
                               name=f"p1{c}_{h}_{g}")
                nc.gpsimd.tensor_scalar(p1t[:], S[:], 0.5 * SM * SM, SM,
                                        op0=ALU.mult, op1=ALU.add)
                nc.gpsimd.scalar_tensor_tensor(
                    et[:], p1t[:], 1.0, S[:], op0=ALU.mult, op1=ALU.mult)
            else:
                nc.scalar.activation(et[:], S[:], AF.Exp, scale=SM)
            etiles[(h, g)] = et

        def emit_av_group(c, h, g):
            if h % 2 == 0 and g == 0:
                avT[0] = p_av.tile([P, 2, QC], f32, tag="av",
                                   name=f"av{c}_{h}")
            po = avT[0][0:65, h % 2, :]
            et = etiles.pop((h, g))
            for jj in range(2):
                kp = 2 * g + jj
                nc.tensor.matmul(
                    po,
                    lhsT=vA[:, 2 * kp:2 * kp + 2, h * 65:h * 65 + 65],
                    rhs=et[:, 2 * jj:2 * jj + 2, :],
                    start=(kp == 0 and h % 2 == 0),
                    stop=(kp == 7 and h % 2 == 1), perf_mode=DR)

        def emit_head_evac(c, h):
            hp, hh = h // 2, h % 2
            po = avT[0][0:65, h % 2, :]
            rec = p_r.tile([1, QC], f32, tag="rec", name=f"rec{c}_{h}")
            if h in pool_set:
                den = p_r.tile([1, QC], f32, tag="den", name=f"den{c}_{h}")
                nc.vector.tensor_scalar_add(den[:], po[64:65, :], 1024.0)
                nc.vector.reciprocal(rec[:], den[:])
            else:
                nc.vector.reciprocal(rec[:], po[64:65, :])
            bc = p_r.tile([64, QC], f32, tag="bc", name=f"bc{c}_{h}")
            nc.gpsimd.partition_broadcast(bc[:], rec[:])
            dst = oT[c % 2][64 * hh:64 * hh + 64, hp, :]
            if h in pool_set:
                nc.vector.scalar_tensor_tensor(
                    dst, po[0:64, :], vsum_sb[0:64, h:h + 1], bc[:],
                    op0=ALU.add, op1=ALU.mult)
            else:
                nc.vector.tensor_tensor(dst, po[0:64, :], bc[:], ALU.mult)

        # ---- FFN for chunk u, emitted as fine-grained quanta ----
        def gen_oproj(c, qsl):
            """O-projection + residual + ffn-stats partials for chunk c."""
            ot = oT[c % 2]
            xr = p_xr.tile([P, 8, QC], f32, tag="xr", name=f"xr{c}")
            nc.sync.dma_start(xr[:], xbT_v[:, :, qsl])
            red = p_st.tile([P, QC], f32, tag="red", name=f"red{c}")
            ctx_red[0] = red
            for dp in range(4):  # dm-pairs
                pO = pools["dn"].tile([P, 2, QC], f32, tag="dn",
                                      name=f"O{c}_{dp}")
                for i in range(2):
                    dm = 2 * dp + i
                    for j in range(4):
                        nc.tensor.matmul(
                            pO[:, i, :],
                            lhsT=wo_sb[:, 2 * j:2 * j + 2,
                                       dm * P:(dm + 1) * P],
                            rhs=ot[:, 2 * j:2 * j + 2, :],
                            start=(i == 0 and j == 0),
                            stop=(i == 1 and j == 3), perf_mode=DR)
                    yield 'pe'
                for i in range(2):
                    dm = 2 * dp + i
                    t1 = p_yt.tile([P, QC], f32, tag="yt",
                                   name=f"t1{c}_{dp}_{i}")
                    nc.vector.tensor_scalar_add(t1[:], pO[:, i, :],
                                                wbo_sb[:, dm:dm + 1])
                    # x2 = x + a_alpha * o_proj
                    nc.vector.scalar_tensor_tensor(
                        x2[:, dm, :], t1[:], mods[:, 16 + dm:17 + dm],
                        xr[:, dm, :], op0=ALU.mult, op1=ALU.add)
                # ffn-stats partial for this dm-pair
                sqp = p_st.tile([P, 2, QC], bf16, tag="sqp",
                                name=f"sqp{c}_{dp}")
                nc.vector.tensor_tensor(sqp[:], x2[:, 2 * dp:2 * dp + 2, :],
                                        x2[:, 2 * dp:2 * dp + 2, :], ALU.mult)
                redp = p_st.tile([P, QC], f32, tag="redp",
                                 name=f"redp{c}_{dp}")
                nc.vector.tensor_reduce(redp[:],
                                        sqp.rearrange("p o t -> p t o"),
                                        AX.X, ALU.add)
                if dp == 0:
                    nc.vector.tensor_copy(red[:], redp[:])
                else:
                    nc.vector.tensor_tensor(red[:], red[:], redp[:], ALU.add)
                yield 'lite'

        def gen_ffnnorm(c):
            """ffn AdaRMSNorm for chunk c (rsqrt via DVE Newton)."""
            red = ctx_red[0]
            mred = p_st.tile([P, QC], f32, tag="mred", name=f"mred{c}")
            nc.gpsimd.partition_all_reduce(mred[:], red[:], channels=P,
                                           reduce_op=RADD)
            # 2 Newton iterations for rsqrt(mred/D + eps), y0 = 1
            y1 = p_r.tile([P, QC], f32, tag="y1", name=f"y1{c}")
            nc.vector.tensor_scalar(y1[:], mred[:], -0.5 / D,
                                    1.5 - 0.5 * EPS,
                                    op0=ALU.mult, op1=ALU.add)
            ya = p_r.tile([P, QC], f32, tag="ya", name=f"ya{c}")
            nc.vector.tensor_tensor(ya[:], y1[:], y1[:], ALU.mult)
            yb = p_r.tile([P, QC], f32, tag="yb", name=f"yb{c}")
            nc.vector.scalar_tensor_tensor(yb[:], mred[:], -0.5 / D, ya[:],
                                           op0=ALU.mult, op1=ALU.mult)
            nc.vector.tensor_scalar_add(yb[:], yb[:], 1.5)
            rbc2 = p_r.tile([P, QC], f32, tag="rbc2", name=f"rbc2{c}")
            nc.vector.tensor_tensor(rbc2[:], y1[:], yb[:], ALU.mult)
            yield 'lite'
            for o in range(8):
                nc.vector.scalar_tensor_tensor(
                    xn2[:, o, :], x2[:, o, :], mods[:, 24 + o:25 + o],
                    rbc2[:], op0=ALU.mult, op1=ALU.mult)
                nc.vector.tensor_scalar_add(xn2[:, o, :], xn2[:, o, :],
                                            mods[:, 32 + o:33 + o])
                if o == 3:
                    yield 'lite'
            yield 'lite'

        def gen_up_pair(c, pr, wh_t):
            """SwiGLU up + silu-via-exp for mi pair (2pr, 2pr+1)."""
            upT = pools["up"].tile([P, 4, QC], f32, tag="up",
                                   name=f"up{c}_{pr}")
            for w_sb, base in ((wg_sb, 0), (wh_t, 2)):
                for i in range(2):
                    mi = 2 * pr + i
                    wsl = (slice(mi * P, (mi + 1) * P) if w_sb is wg_sb
                           else slice(i * P, (i + 1) * P))
                    for j in range(4):
                        nc.tensor.matmul(
                            upT[:, base + i, :],
                            lhsT=w_sb[:, 2 * j:2 * j + 2, wsl],
                            rhs=xn2[:, 2 * j:2 * j + 2, :],
                            start=(i == 0 and j == 0),
                            stop=(i == 1 and j == 3), perf_mode=DR)
                    yield 'pe'
            # silu(g)*h = g*h / (1 + e^-g)
            eg = p_sg.tile([P, 2, QC], bf16, tag="eg", name=f"eg{c}_{pr}")
            if pools["pool_mult"]:
                # drain: up psum is multi-buffered; read it directly
                nc.scalar.activation(eg[:], upT[:, 0:2, :], AF.Exp,
                                     scale=-1.0)
                nc.vector.tensor_scalar_add(eg[:], eg[:], 1.0)
                with nc.allow_low_precision(reason="silu sigmoid in bf16"):
                    nc.vector.reciprocal(eg[:], eg[:])
                u = p_sg.tile([P, 2, QC], bf16, tag="gh", name=f"u{c}_{pr}")
                nc.vector.tensor_tensor(u[:], upT[:, 0:2, :], eg[:],
                                        ALU.mult)
                nc.vector.scalar_tensor_tensor(
                    m_sb[:, 2 * pr:2 * pr + 2, :], upT[:, 2:4, :], 1.0,
                    u[:], op0=ALU.mult, op1=ALU.mult)
                yield 'lite'
            else:
                # evacuate psum first (frees the up tile for the next pair)
                gha = p_sg.tile([P, 4, QC], bf16, tag="gha",
                                name=f"gha{c}_{pr}")
                nc.vector.tensor_copy(gha[:], upT[:])
                yield 'lite'
                nc.scalar.activation(eg[:], gha[:, 0:2, :], AF.Exp,
                                     scale=-1.0)
                nc.vector.tensor_scalar_add(eg[:], eg[:], 1.0)
                with nc.allow_low_precision(reason="silu sigmoid in bf16"):
                    nc.vector.reciprocal(eg[:], eg[:])
                # m = (g * sigmoid(g)) * h, all operands in SBUF
                u = p_sg.tile([P, 2, QC], bf16, tag="gh", name=f"u{c}_{pr}")
                nc.vector.tensor_tensor(u[:], gha[:, 0:2, :], eg[:],
                                        ALU.mult)
                nc.gpsimd.tensor_tensor(m_sb[:, 2 * pr:2 * pr + 2, :],
                                        gha[:, 2:4, :], u[:], ALU.mult)
                yield 'lite'

        def gen_down(c, dp, qsl):
            """down-proj for dm pair (2dp, 2dp+1) of chunk c + y out."""
            pD = pools["dn"].tile([P, 2, QC], f32, tag="dn",
                                  name=f"D{c}_{dp}")
            for i in range(2):
                dm = 2 * dp + i
                for mp in range(16):
                    nc.tensor.matmul(
                        pD[:, i, :],
                        lhsT=wout_sb[:, 2 * mp:2 * mp + 2,
                                     dm * P:(dm + 1) * P],
                        rhs=m_sb[:, 2 * mp:2 * mp + 2, :],
                        start=(i == 0 and mp == 0), stop=False, perf_mode=DR)
                    if mp % 4 == 3:
                        yield 'pe'
                nc.tensor.matmul(
                    pD[:, i, :], lhsT=outb_row[0:1, dm * P:(dm + 1) * P],
                    rhs=ones8r[:], start=False, stop=(i == 1))
                yield 'pe'
            for i in range(2):
                dm = 2 * dp + i
                yt = p_yt.tile([P, QC], f32, tag="yt", name=f"yt{c}_{dp}_{i}")
                nc.vector.scalar_tensor_tensor(
                    yt[:], pD[:, i, :], mods[:, 40 + dm:41 + dm],
                    x2[:, dm, :], op0=ALU.mult, op1=ALU.add)
                nc.sync.dma_start(y_v[:, dm, qsl], yt[:])
            yield 'lite'

        ctx_red = [None]
        from collections import deque
        gens = deque()

        def pull(budget=1):
            lite_run = 0
            while gens and budget > 0:
                try:
                    tag = next(gens[0])
                except StopIteration:
                    gens.popleft()
                    continue
                if tag == 'pe':
                    budget -= 1
                else:
                    lite_run += 1
                    if lite_run >= 3:
                        budget -= 1

        def emit_attn_window(a, qsl_a):
            for h in range(NH):
                for g in range(4):
                    if h >= 1:
                        emit_av_group(a, h - 1, g)
                    pull()
                    emit_score_group(a, h, g, qsl_a)
                    pull()
                if h >= 2 and h % 2 == 0:
                    emit_head_evac(a, h - 2)
                    emit_head_evac(a, h - 1)
            for g in range(4):
                emit_av_group(a, NH - 1, g)
                pull()
            emit_head_evac(a, NH - 2)
            emit_head_evac(a, NH - 1)

        # ---- window 0: chunk-0 attention ----
        emit_attn_window(0, slice(0, QC))

        x2 = chp.tile([P, 8, QC], bf16, name="x2")
        xn2 = chp.tile([P, 8, QC], f8, name="xn2")
        m_sb = chp.tile([P, 32, QC], f8, name="m_sb")
        p_sg = tc.alloc_tile_pool(name="p_sg", bufs=2)
        p_yt = tc.alloc_tile_pool(name="p_yt", bufs=3)
        p_st = tc.alloc_tile_pool(name="p_st", bufs=1)
        p_wh = tc.alloc_tile_pool(name="p_wh", bufs=2)
        p_xr = tc.alloc_tile_pool(name="p_xr", bufs=2)
        pools = {"up": tc.alloc_tile_pool(name="p_up", bufs=1, space="PSUM"),
                 "dn": tc.alloc_tile_pool(name="p_dn", bufs=1, space="PSUM"),
                 "pool_mult": False}

        for w in range(1, NQC + 1):
            a, u = w, w - 1  # attention chunk / FFN chunk this window
            qsl_a = slice((a % NQC) * QC, (a % NQC + 1) * QC)
            qsl_u = slice(u * QC, (u + 1) * QC)
            wh_tiles = []
            for pr in range(16):
                wt = p_wh.tile([P, 8, 2 * P], f8, tag="wh",
                               name=f"wh{w}_{pr}")
                nc.sync.dma_start(
                    wt[:], wh_v[:, :, 2 * pr * P:(2 * pr + 2) * P])
                wh_tiles.append(wt)
            if a == NQC:
                # drain: rebuild up/dn pools with double buffering
                pools["dn"].release()
                pools["up"].release()
                p_av.release()
                p_sc.release()
                pools["up"] = tc.alloc_tile_pool(name="p_up2", bufs=3,
                                                 space="PSUM")
                pools["dn"] = tc.alloc_tile_pool(name="p_dn2", bufs=2,
                                                 space="PSUM")
                pools["pool_mult"] = True
            gens.append(gen_oproj(u, qsl_u))
            gens.append(gen_ffnnorm(u))
            for pr in range(16):
                gens.append(gen_up_pair(u, pr, wh_tiles[pr]))
            for dp in range(4):
                gens.append(gen_down(u, dp, qsl_u))
            if a < NQC:
                emit_attn_window(a, qsl_a)
                while gens:
                    pull(budget=100)
            else:
                # drain window: free attention psum, double-buffer the ups
                while gens:
                    pull(budget=100)

        pools["dn"].release()
        pools["up"].release()
        for pool in (p_xr, p_wh, p_st, p_yt, p_sg, p_r, p_e,
                     chp, persW, persA):
            pool.release()
        if p_q is not None:
            p_q.release()

    nc.compile()
    return nc


def _get_nc():
    if "nc" not in _CACHE:
        _CACHE["nc"] = _build_nc()
    return _CACHE["nc"]


def make_in_maps(x, t, attn_gamma_w, attn_beta_w, W_q, W_k, W_v, W_o,
                 attn_alpha_w, ffn_gamma_w, ffn_beta_w, gate_w, hidden_w,
                 out_w, out_b, ffn_alpha_w):
    import ml_dtypes
    bf = ml_dtypes.bfloat16
    f8 = ml_dtypes.float8_e4m3
    f32 = np.float32

    def T8(a):
        return np.ascontiguousarray(np.asarray(a, f32).T).astype(f8)

    xT = np.ascontiguousarray(np.asarray(x, f32).transpose(0, 2, 1))
    t = np.asarray(t, f32)
    modw = np.ascontiguousarray(np.concatenate(
        [np.asarray(w, f32) for w in (attn_gamma_w, attn_beta_w, attn_alpha_w,
                                      ffn_gamma_w, ffn_beta_w, ffn_alpha_w)],
        axis=0).T).astype(bf)                          # [256, 6144]
    shared = {
        "modw": modw,
        "wq": T8(W_q), "wk": T8(W_k), "wv": T8(W_v), "wo": T8(W_o),
        "wg": T8(gate_w), "wh": T8(hidden_w), "wout": T8(out_w),
        "outbr": np.ascontiguousarray(
            np.asarray(out_b, f32).reshape(1, D)).astype(bf),
    }
    in_maps = []
    for c in range(NCORES):
        b, h = c // 2, c % 2
        if h == 0:
            xbT = xT[b]
        else:
            xbT = np.concatenate([xT[b][:, LOWN:], xT[b][:, :LOWN]], axis=1)
        in_maps.append(dict(
            shared,
            xbT=np.ascontiguousarray(xbT),
            tb=np.ascontiguousarray(t[b].reshape(2, P).T).astype(bf),
        ))
    return in_maps


def kernel(**inputs):
    from concourse.bass_utils import run_bass_kernel_spmd

    nc = _get_nc()
    in_maps = make_in_maps(**inputs)
    res = run_bass_kernel_spmd(nc, in_maps, core_ids=list(range(NCORES)))
    x = np.asarray(inputs["x"])
    yfull = np.empty((x.shape[0], L, D), dtype=np.float32)
    for c in range(NCORES):
        b, h = c // 2, c % 2
        yfull[b, h * LOWN:(h + 1) * LOWN, :] = res.results[c]["y"].T
    return yfull
